# revision 63
# baseline (speedup 1.0000x reference)
"""Causal self-attention with RoPE on 8 Trainium2 NeuronCores.

Problem (hardcoded): x (4, 2048, 2048) f32, w_attn (2048, 6144),
w_proj (2048, 2048), rope_cos/rope_sin (2048, 64), 16 heads, hd=128.

Sharding: 8 cores = 4 batches x 2 head-groups (8 heads each).  Each core
computes qkv projection for its heads, RoPE, causal attention, and a
partial output projection (its head-group's rows of w_proj).  The host
sums the two partials per batch (the "all-reduce after c_proj") and
transposes back, since the device kernel works fully transposed.

Device layout choices:
  - All matmul operands staged in bf16 (same PE rate as f32r, half the
    DMA/SBUF): x^T resident in SBUF, weights loaded once, roped k and v
    kept resident in SBUF through attention (no DRAM roundtrip); roped
    q roundtrips through DRAM.  PSUM accumulation is always f32.
  - qT, kT stored [hd=128 partitions, T free]; S^T tiles [j_keys, q]
    come straight from matmul(lhsT=kT_j, rhs=qT_q).  Softmax exp is
    elementwise (no max subtraction needed: scores ~ N(0,1), max ~ 6);
    causality = skipping j>q blocks + a -1e9 bias matmul on diagonal
    blocks (exp underflows to 0, so no vector-engine mask).  The PV
    matmul consumes P^T directly with v in natural [T, hd] layout as
    lhsT, producing o^T with no transposes anywhere.
  - softmax denominator accumulated on the PE (ones-column matmul into
    a PSUM bank across J tiles); reciprocal runs on DVE in 4 chunks of
    [1,128] pipelined with 4 chunk-broadcast matmuls; the whole
    normalization tail for chunk Q is emitted during chunk Q+1 so the
    in-order PE queue never waits on it.
  - RoPE pairs (2i, 2i+1) are host-permuted to (i, 64+i) by permuting
    w_attn's q/k columns (dot products are permutation invariant), so
    the rotation acts on contiguous partition ranges.
"""

import sys

sys.path.insert(0, "/opt/trn_rl_repo")

import numpy as np

import concourse.bass as bass
import concourse.mybir as mybir
import concourse.tile as tile

F32 = mybir.dt.float32
F32R = mybir.dt.float32r
BF16 = mybir.dt.bfloat16
P = 128


# --------------------------------------------------------------------------
# This container's walrus build rejects any instruction carrying more than
# one sem wait.  Split extras onto NoOps inserted before the instruction on
# the same engine (per-engine program order makes the waits complete first).
def _split_multi_waits(nc):
    n = 0
    for fn in nc.m.functions:
        for bb in fn.blocks:
            out = []
            changed = False
            for inst in bb.instructions:
                si = inst.sync_info
                waits = list(si.on_wait or []) if si is not None else []
                if len(waits) > 1:
                    changed = True
                    n += 1
                    for w in waits[:-1]:
                        nop = mybir.InstNoOp(
                            name=nc.get_next_instruction_name(),
                            engine=inst.engine,
                            ins=[],
                            outs=[],
                            sync_info=mybir.SyncInfo(on_wait=[w], on_update=[]),
                        )
                        try:
                            nc.register_instruction(nop, overwrite=True)
                        except Exception:
                            pass
                        out.append(nop)
                    inst.sync_info = mybir.SyncInfo(
                        on_wait=[waits[-1]], on_update=list(si.on_update or [])
                    )
                out.append(inst)
            if changed:
                bb.instructions = out
    return n


def build_attention_core(T=2048, C=2048, G=8):
    """One core's program.  T tokens, C model dim, G heads in this core's
    group (hd=128 each).  Returns the Bass object."""
    KO = C // P          # contraction tiles over model dim
    QW = 512             # query chunk width
    NQ = T // QW         # 4 query chunks
    JPQ = QW // P        # j tiles per q chunk width
    NJ = T // P          # total j tiles
    NTC = T // QW        # 512-wide t chunks (qk projection)
    VN = 512             # v column chunk
    NV = (G * P) // VN   # 2
    NTB = T // P         # 128-tall t blocks (v projection)

    nc = bass.Bass()
    xt = nc.dram_tensor(
        "xt", [T // 512, P, KO, 512], BF16, kind="ExternalInput"
    )
    wqk = nc.dram_tensor("wqk", [2 * G, P, KO, P], BF16, kind="ExternalInput")
    wv = nc.dram_tensor("wv", [NV, P, KO, VN], BF16, kind="ExternalInput")
    wp = nc.dram_tensor("wp", [P, KO, G, P], BF16, kind="ExternalInput")
    # cosd = [cos; cos], sind = [-sin; +sin]  (rope = raw*cosd + swapped*sind)
    cosp = nc.dram_tensor("cosp", [P, T], BF16, kind="ExternalInput")
    sinp = nc.dram_tensor("sinp", [P, T], BF16, kind="ExternalInput")
    maskt = nc.dram_tensor("maskt", [P, P], F32R, kind="ExternalInput")
    identt = nc.dram_tensor("identt", [P, P], F32R, kind="ExternalInput")
    ones_c = nc.dram_tensor("ones_c", [P, 1], BF16, kind="ExternalInput")
    ones1 = nc.dram_tensor("ones1", [1, P], F32R, kind="ExternalInput")
    outT = nc.dram_tensor("outT", [C, T], F32, kind="ExternalOutput")

    scale = 1.0 / np.sqrt(128.0)

    with tile.TileContext(nc) as tc:
        with (
            tc.tile_pool(name="dram", bufs=1, space="DRAM") as dram,
            tc.tile_pool(name="const", bufs=1) as cpool,
        ):
            # per-q-chunk od tiles so phase C's chunk-t read only depends on
            # chunk-t writes; [p, h, t] layout puts the strided cost on the
            # 32 small hidden writes so the C-entry read is contiguous
            od = dram.tile([NQ, P, G, QW], BF16)

            with (
                tc.tile_pool(name="xts", bufs=1) as xts_pool,
                tc.tile_pool(name="kall", bufs=1) as kall_pool,
                tc.tile_pool(name="vall", bufs=1) as va_pool,
            ):
                # resident through phases A and B (all bf16):
                # x^T [ki, ko, t]; roped k [hd, h, t];
                # v_all[ti, to, hh*128+d] = v[to*128+ti, head hh, d]
                # x chunk 0 goes first (the v matmuls need it); consts and
                # the rest alternate between the sync and gpsimd DMA queues
                xtc = [
                    xts_pool.tile([P, KO, QW], BF16, tag=f"xts{c}",
                                  name=f"xts{c}")
                    for c in range(NTC)
                ]
                nc.sync.dma_start(xtc[0][:], xt[0])
                cos_s = cpool.tile([P, T], BF16)
                nc.gpsimd.dma_start(cos_s[:], cosp[:])
                sin_s = cpool.tile([P, T], BF16)
                nc.gpsimd.dma_start(sin_s[:], sinp[:])
                mask_s = cpool.tile([P, P], F32R)
                nc.gpsimd.dma_start(mask_s[:], maskt[:])
                ident_s = cpool.tile([P, P], F32R)
                nc.gpsimd.dma_start(ident_s[:], identt[:])
                one_col = cpool.tile([P, 1], BF16)
                nc.gpsimd.dma_start(one_col[:], ones_c[:])
                one_row = cpool.tile([1, P], F32R)
                nc.gpsimd.dma_start(one_row[:], ones1[:])
                nc.sync.dma_start(xtc[1][:], xt[1])
                nc.gpsimd.dma_start(xtc[2][:], xt[2])
                nc.gpsimd.dma_start(xtc[3][:], xt[3])
                kall = kall_pool.tile([P, G, T], BF16, tag="kall")
                v_all = va_pool.tile([P, NJ, G * P], BF16, tag="vall")

                # --- v (kept in SBUF, natural [t, d] layout) ---
                with (
                    tc.tile_pool(name="wv", bufs=2) as wv_pool,
                    tc.tile_pool(name="psV", bufs=2, space="PSUM") as psV,
                ):
                    for n2 in range(NV):
                        wv_s = wv_pool.tile([P, KO, VN], BF16, tag="wv")
                        nc.scalar.dma_start(wv_s[:], wv[n2])
                        for tb in range(NTB):
                            psv = psV.tile([P, VN], F32, tag="pv")
                            for kc in range(KO):
                                nc.tensor.matmul(
                                    psv[:],
                                    xtc[tb // JPQ][
                                        :, kc,
                                        (tb % JPQ) * P : (tb % JPQ + 1) * P,
                                    ],
                                    wv_s[:, kc, :],
                                    start=(kc == 0),
                                    stop=(kc == KO - 1),
                                )
                            nc.scalar.activation(
                                v_all[:, tb, n2 * VN : (n2 + 1) * VN],
                                psv[:],
                                mybir.ActivationFunctionType.Copy,
                            )

                # --- fused qk projection + RoPE + attention ---
                # per head h: emit qk matmuls+rope for head h, then the
                # attention for head h-1 (1-head lag keeps every PE input
                # ready before the in-order PE queue reaches it).
                with (
                    tc.tile_pool(name="wqk", bufs=2) as wqk_pool,
                    tc.tile_pool(name="qkraw", bufs=1) as qkraw_pool,
                    tc.tile_pool(name="ropf", bufs=2) as ropf_pool,
                    tc.tile_pool(name="sw", bufs=2) as sw_pool,
                    tc.tile_pool(name="ropb", bufs=3) as ropb_pool,
                    tc.tile_pool(name="pt", bufs=5) as pt_pool,
                    tc.tile_pool(name="racc", bufs=2) as racc_pool,
                    tc.tile_pool(name="rsb", bufs=2) as rsb_pool,
                    tc.tile_pool(name="rq", bufs=2) as rq_pool,
                    tc.tile_pool(name="rinv", bufs=2) as rinv_pool,
                    tc.tile_pool(name="oacc", bufs=2) as oacc_pool,
                    tc.tile_pool(name="psSh", bufs=4, space="PSUM") as psSh,
                    tc.tile_pool(name="psO", bufs=3, space="PSUM") as psO,
                    tc.tile_pool(name="psR", bufs=1, space="PSUM") as psR,
                ):
                    qts = {}

                    def emit_qk(m):
                        h, is_k = m // 2, m % 2
                        w_s = wqk_pool.tile([P, KO, P], BF16, tag="wqk")
                        nc.sync.dma_start(w_s[:], wqk[m])
                        raw = qkraw_pool.tile([P, T], BF16, tag="qkraw")
                        ropf = ropf_pool.tile([P, T], BF16, tag="ropf")
                        for c in range(NTC):
                            sl = slice(c * QW, (c + 1) * QW)
                            pss = psSh.tile([P, QW], F32, tag="ps")
                            for kc in range(KO):
                                nc.tensor.matmul(
                                    pss[:],
                                    w_s[:, kc, :],
                                    xtc[c][:, kc, :],
                                    start=(kc == 0),
                                    stop=(kc == KO - 1),
                                    skip_group_check=True,
                                )
                            nc.vector.tensor_copy(raw[:, sl], pss[:])
                            nc.vector.tensor_mul(
                                ropf[:, sl], pss[:], cos_s[:, sl]
                            )
                        # RoPE: rope = raw*[cos;cos] + swap(raw)*[-sin;+sin]
                        sw = sw_pool.tile([P, T], BF16, tag="sw")
                        nc.sync.dma_start(sw[0:64, :], raw[64:128, :])
                        nc.sync.dma_start(sw[64:128, :], raw[0:64, :])
                        nc.vector.tensor_mul(sw[:], sw[:], sin_s[:])
                        if is_k:
                            nc.vector.tensor_add(kall[:, h, :], ropf[:], sw[:])
                        else:
                            rb = ropb_pool.tile([P, T], BF16, tag="ropb")
                            nc.vector.tensor_add(rb[:], ropf[:], sw[:])
                            qts[h] = rb

                    # Normalization tail, three PE-async stages: the rowsum
                    # matmul + DVE reciprocal fire early in the next chunk
                    # (inputs long since ready); the broadcast matmul +
                    # scale + od write fire >=4us of PE work later so the
                    # in-order PE queue never waits on the reciprocal.
                    rs_pend = []
                    norm_pend = []

                    def flush_rs(direct=False):
                        racc_p, pso_p, oT_p, hp, Qp = rs_pend.pop(0)
                        psr = psR.tile([1, QW], F32, tag="psr")
                        nc.tensor.matmul(
                            psr[:], one_col[:], racc_p[:],
                            start=True, stop=True, skip_group_check=True,
                        )
                        rinv1 = rsb_pool.tile([1, QW], F32R, tag="rinv1")
                        if direct:
                            # drain path: shortest latency, DVE cost is moot
                            with nc.allow_low_precision(reason="f32r is 4B"):
                                nc.vector.reciprocal(rinv1[:], psr[:])
                        else:
                            # reciprocal is free-size-bound (~6.5 cyc/elem/
                            # lane): fold [1,512] -> [128,4] via SBUF-SBUF
                            # DMA so it runs on 4 elems/lane (161ns) instead
                            # of 512 (3.3us)
                            rsb = rsb_pool.tile([1, QW], F32, tag="rsb")
                            nc.vector.tensor_copy(rsb[:], psr[:])
                            rq = rq_pool.tile([P, QW // P], F32, tag="rq")
                            nc.sync.dma_start(rq[:], rsb[:])
                            rqr = rq_pool.tile([P, QW // P], F32R, tag="rqr")
                            with nc.allow_low_precision(reason="f32r is 4B"):
                                nc.vector.reciprocal(rqr[:], rq[:])
                            nc.sync.dma_start(rinv1[:], rqr[:])
                        norm_pend.append((rinv1, pso_p, oT_p, hp, Qp))

                    def flush_norm():
                        rinv1, pso_p, oT_p, hp, Qp = norm_pend.pop(0)
                        psrep = psSh.tile([P, QW], F32, tag="ps")
                        nc.tensor.matmul(
                            psrep[:], one_row[:], rinv1[:],
                            start=True, stop=True, skip_group_check=True,
                        )
                        rinv = rinv_pool.tile([P, QW], F32, tag="rinv")
                        nc.scalar.activation(
                            rinv[:], psrep[:],
                            mybir.ActivationFunctionType.Copy,
                        )
                        sl = slice(Qp * QW, (Qp + 1) * QW)
                        nc.vector.tensor_mul(oT_p[:, sl], pso_p[:], rinv[:])
                        nc.scalar.dma_start(od[Qp][:, hp, :], oT_p[:, sl])

                    def emit_attn(h):
                        qT = qts.pop(h)
                        oT = oacc_pool.tile([P, T], BF16, tag="oacc")
                        for Q in range(NQ):
                            jmax = JPQ * (Q + 1) - 1  # inclusive
                            pso = psO.tile([P, QW], F32, tag="pso")
                            racc = racc_pool.tile([P, QW], BF16, tag="racc")
                            # PV(J) consumes exp(S(J)) from ACT; emit it LOOK
                            # S-matmuls later so PE never stalls on ACT.
                            LOOK = 3
                            pend = []
                            for J in range(jmax + 1 + LOOK):
                                if J <= jmax:
                                    k_d = J - JPQ * Q  # diag idx if >= 0
                                    co = max(k_d, 0) * P
                                    pss = psSh.tile([P, QW], F32, tag="ps")
                                    nc.tensor.matmul(
                                        pss[:, co:],
                                        kall[:, h, J * P : (J + 1) * P],
                                        qT[:, Q * QW + co : (Q + 1) * QW],
                                        start=True,
                                        stop=(k_d < 0),
                                        skip_group_check=True,
                                    )
                                    if k_d >= 0:
                                        # causal mask: -1e9 bias, exp -> 0
                                        nc.tensor.matmul(
                                            pss[:, co : co + P],
                                            mask_s[:],
                                            ident_s[:],
                                            start=False,
                                            stop=True,
                                            skip_group_check=True,
                                        )
                                    pT = pt_pool.tile([P, QW], BF16, tag="pt")
                                    nc.scalar.activation(
                                        pT[:, co:], pss[:, co:],
                                        mybir.ActivationFunctionType.Exp,
                                        scale=scale,
                                    )
                                    # rowsum on DVE (bf16): r += exp tile
                                    if J == 0:
                                        nc.vector.tensor_copy(racc[:], pT[:])
                                    else:
                                        nc.vector.tensor_add(
                                            racc[:, co:], racc[:, co:],
                                            pT[:, co:],
                                        )
                                    pend.append((J, co, pT))
                                if J >= LOOK:
                                    Jp, cop, pTp = pend.pop(0)
                                    nc.tensor.matmul(
                                        pso[:, cop:],
                                        v_all[:, Jp, h * P : (h + 1) * P],
                                        pTp[:, cop:],
                                        start=(Jp == 0),
                                        stop=(Jp == jmax),
                                        skip_group_check=True,
                                    )
                                if J == 5 and rs_pend:
                                    flush_rs()
                                if J in (10, 14) and norm_pend:
                                    flush_norm()
                            rs_pend.append((racc, pso, oT, h, Q))

                    for h in range(G):
                        emit_qk(2 * h)
                        # head-boundary rowsum flush lands here: the PE
                        # reaches it a full qk-head after the racc chain of
                        # the previous head finished, so it never stalls
                        if rs_pend:
                            flush_rs()
                        emit_qk(2 * h + 1)
                        if h >= 1:
                            emit_attn(h - 1)
                    emit_attn(G - 1)
                    while rs_pend:
                        flush_rs(direct=True)
                    while norm_pend:
                        flush_norm()

                # ------------- Phase C: output projection -------------
                with (
                    tc.tile_pool(name="wp", bufs=1) as wp_pool,
                    tc.tile_pool(name="otc", bufs=2) as otc_pool,
                    tc.tile_pool(name="csb", bufs=4) as csb_pool,
                    tc.tile_pool(name="psC", bufs=4, space="PSUM") as psC,
                ):
                    # single contiguous DMA: the tile's readers wait its whole
                    # writer set, so one fast transfer beats 16 chunked ones
                    wp_s = wp_pool.tile([P, KO, G, P], BF16, tag="wp")
                    nc.gpsimd.dma_start(wp_s[:], wp[:])
                    for t in range(NQ):
                        oTt = otc_pool.tile([P, G, QW], BF16, tag="otc")
                        nc.sync.dma_start(
                            oTt[:],
                            od[t],
                        )
                        for m in range(KO):
                            psc = psC.tile([P, QW], F32, tag="psc")
                            for h in range(G):
                                nc.tensor.matmul(
                                    psc[:],
                                    wp_s[:, m, h, :],
                                    oTt[:, h, :],
                                    start=(h == 0),
                                    stop=(h == G - 1),
                                )
                            csb = csb_pool.tile([P, QW], F32, tag="csb")
                            nc.any.tensor_copy(csb[:], psc[:])
                            nc.sync.dma_start(
                                outT[
                                    m * P : (m + 1) * P,
                                    t * QW : (t + 1) * QW,
                                ],
                                csb[:],
                            )

    _split_multi_waits(nc)
    return nc


# --------------------------------------------------------------------------
def _prep_core_inputs(xb, w_attn, w_proj, rope_cos, rope_sin, g, G=8):
    """Host-side shard prep for one core: batch slice xb (T, C), group g."""
    from ml_dtypes import bfloat16

    T, C = xb.shape
    KO = C // P
    VN = 512
    NV = (G * P) // VN
    gc = g * G * P  # column offset of this group within one qkv section

    # x^T arranged [chunk, ki, ko, t-in-chunk]: each 512-token chunk is
    # contiguous in DRAM so the chunk DMA moves 16KB lines per partition
    xtT = np.ascontiguousarray(xb.T)  # (C, T)
    xt = np.ascontiguousarray(
        xtT.reshape(KO, P, T // 512, 512).transpose(2, 1, 0, 3)
    )

    # q,k columns for this group, RoPE pair-permuted (2i,2i+1) -> (i,64+i),
    # interleaved m=2h -> q head h, m=2h+1 -> k head h
    perm = np.empty(P, dtype=np.int64)
    perm[:64] = np.arange(0, P, 2)
    perm[64:] = np.arange(1, P, 2)
    wq = w_attn[:, gc : gc + G * P].reshape(C, G, P)[:, :, perm]
    wk = w_attn[:, C + gc : C + gc + G * P].reshape(C, G, P)[:, :, perm]
    wqk_cols = np.stack([wq, wk], axis=2).reshape(C, 2 * G * P)
    wqk = np.ascontiguousarray(
        wqk_cols.reshape(KO, P, 2 * G, P).transpose(2, 1, 0, 3)
    )

    wv_cols = w_attn[:, 2 * C + gc : 2 * C + gc + G * P]  # (C, G*128)
    wv = np.ascontiguousarray(
        wv_cols.reshape(KO, P, NV, VN).transpose(2, 1, 0, 3)
    )

    wp_rows = w_proj[gc : gc + G * P, :]  # (G*128, C)
    wp = np.ascontiguousarray(
        wp_rows.reshape(G, P, KO, P).transpose(1, 2, 0, 3)
    )

    cT = rope_cos[:T].T  # (64, T)
    sT = rope_sin[:T].T
    cospT = np.ascontiguousarray(np.concatenate([cT, cT], axis=0))  # (128, T)
    sinpT = np.ascontiguousarray(np.concatenate([-sT, sT], axis=0))
    # -1e9 bias on the strictly-lower (key > query) triangle of the S^T
    # diagonal block, delivered as lhsT of a bias matmul against identity:
    # pss[jj,qq] += maskb[qq,jj] -> maskb = strict upper triangle.
    maskb = -1e9 * np.triu(np.ones((P, P), dtype=np.float32), k=1)

    return {
        "xt": xt.astype(bfloat16),
        "wqk": wqk.astype(bfloat16),
        "wv": wv.astype(bfloat16),
        "wp": wp.astype(bfloat16),
        "cosp": cospT.astype(bfloat16),
        "sinp": sinpT.astype(bfloat16),
        "maskt": maskb,
        "identt": np.eye(P, dtype=np.float32),
        "ones_c": np.ones((P, 1), dtype=bfloat16),
        "ones1": np.ones((1, P), dtype=np.float32),
    }


_NC_CACHE = {}
TRACE = False
LAST_RESULTS = None


def kernel(x, w_attn, w_proj, rope_cos, rope_sin):
    from concourse.bass_utils import run_bass_kernel_spmd

    x = np.asarray(x, dtype=np.float32)
    w_attn = np.asarray(w_attn, dtype=np.float32)
    w_proj = np.asarray(w_proj, dtype=np.float32)
    rope_cos = np.asarray(rope_cos, dtype=np.float32)
    rope_sin = np.asarray(rope_sin, dtype=np.float32)

    B, T, C = x.shape
    G = 8  # heads per group (16 heads / 2 groups)

    key = (T, C, G)
    if key not in _NC_CACHE:
        _NC_CACHE[key] = build_attention_core(T=T, C=C, G=G)
    nc = _NC_CACHE[key]

    in_maps = []
    for core in range(8):
        b, g = core // 2, core % 2
        in_maps.append(
            _prep_core_inputs(x[b], w_attn, w_proj, rope_cos, rope_sin, g, G=G)
        )

    res = run_bass_kernel_spmd(nc, in_maps, list(range(8)), trace=TRACE)
    global LAST_RESULTS
    LAST_RESULTS = res

    y = np.empty((B, T, C), dtype=np.float32)
    for b in range(B):
        acc = res.results[2 * b]["outT"] + res.results[2 * b + 1]["outT"]
        y[b] = acc.T
    return y


# revision 66
# speedup vs baseline: 1.1815x; 1.1815x over previous
"""Causal self-attention with RoPE on 8 Trainium2 NeuronCores.

Problem (hardcoded): x (4, 2048, 2048) f32, w_attn (2048, 6144),
w_proj (2048, 2048), rope_cos/rope_sin (2048, 64), 16 heads, hd=128.

Sharding: 8 cores = 4 batches x 2 head-groups (8 heads each).  Each core
computes qkv projection for its heads, RoPE, causal attention, and a
partial output projection (its head-group's rows of w_proj).  The host
sums the two partials per batch (the "all-reduce after c_proj") and
transposes back, since the device kernel works fully transposed.

Device layout choices:
  - All matmul operands staged in bf16 (same PE rate as f32r, half the
    DMA/SBUF): x^T resident in SBUF, weights loaded once, roped k and v
    kept resident in SBUF through attention (no DRAM roundtrip); roped
    q roundtrips through DRAM.  PSUM accumulation is always f32.
  - qT, kT stored [hd=128 partitions, T free]; S^T tiles [j_keys, q]
    come straight from matmul(lhsT=kT_j, rhs=qT_q).  Softmax exp is
    elementwise (no max subtraction needed: scores ~ N(0,1), max ~ 6);
    causality = skipping j>q blocks + a -1e9 bias matmul on diagonal
    blocks (exp underflows to 0, so no vector-engine mask).  The PV
    matmul consumes P^T directly with v in natural [T, hd] layout as
    lhsT, producing o^T with no transposes anywhere.
  - softmax denominator accumulated on the PE (ones-column matmul into
    a PSUM bank across J tiles); reciprocal runs on DVE in 4 chunks of
    [1,128] pipelined with 4 chunk-broadcast matmuls; the whole
    normalization tail for chunk Q is emitted during chunk Q+1 so the
    in-order PE queue never waits on it.
  - RoPE pairs (2i, 2i+1) are host-permuted to (i, 64+i) by permuting
    w_attn's q/k columns (dot products are permutation invariant), so
    the rotation acts on contiguous partition ranges.
"""

import sys

sys.path.insert(0, "/opt/trn_rl_repo")

import numpy as np

import concourse.bass as bass
import concourse.mybir as mybir
import concourse.tile as tile

F32 = mybir.dt.float32
F32R = mybir.dt.float32r
BF16 = mybir.dt.bfloat16
P = 128


# --------------------------------------------------------------------------
# This container's walrus build rejects any instruction carrying more than
# one sem wait.  Split extras onto NoOps inserted before the instruction on
# the same engine (per-engine program order makes the waits complete first).
def _split_multi_waits(nc):
    n = 0
    for fn in nc.m.functions:
        for bb in fn.blocks:
            out = []
            changed = False
            for inst in bb.instructions:
                si = inst.sync_info
                waits = list(si.on_wait or []) if si is not None else []
                if len(waits) > 1:
                    changed = True
                    n += 1
                    for w in waits[:-1]:
                        nop = mybir.InstNoOp(
                            name=nc.get_next_instruction_name(),
                            engine=inst.engine,
                            ins=[],
                            outs=[],
                            sync_info=mybir.SyncInfo(on_wait=[w], on_update=[]),
                        )
                        try:
                            nc.register_instruction(nop, overwrite=True)
                        except Exception:
                            pass
                        out.append(nop)
                    inst.sync_info = mybir.SyncInfo(
                        on_wait=[waits[-1]], on_update=list(si.on_update or [])
                    )
                out.append(inst)
            if changed:
                bb.instructions = out
    return n


def build_attention_core(T=2048, C=2048, G=8):
    """One core's program.  T tokens, C model dim, G heads in this core's
    group (hd=128 each).  Returns the Bass object."""
    KO = C // P          # contraction tiles over model dim
    QW = 512             # query chunk width
    NQ = T // QW         # 4 query chunks
    JPQ = QW // P        # j tiles per q chunk width
    NJ = T // P          # total j tiles
    NTC = T // QW        # 512-wide t chunks (qk projection)
    VN = 512             # v column chunk
    NV = (G * P) // VN   # 2
    NTB = T // P         # 128-tall t blocks (v projection)

    nc = bass.Bass()
    xt = nc.dram_tensor(
        "xt", [T // 512, P, KO, 512], BF16, kind="ExternalInput"
    )
    wqk = nc.dram_tensor("wqk", [2 * G, P, KO, P], BF16, kind="ExternalInput")
    wv = nc.dram_tensor("wv", [NV, P, KO, VN], BF16, kind="ExternalInput")
    wp = nc.dram_tensor("wp", [KO, P, G, P], BF16, kind="ExternalInput")
    # cosd = [cos; cos], sind = [-sin; +sin]  (rope = raw*cosd + swapped*sind)
    cosp = nc.dram_tensor("cosp", [P, T], BF16, kind="ExternalInput")
    sinp = nc.dram_tensor("sinp", [P, T], BF16, kind="ExternalInput")
    maskt = nc.dram_tensor("maskt", [P, P], F32R, kind="ExternalInput")
    identt = nc.dram_tensor("identt", [P, P], F32R, kind="ExternalInput")
    ones_c = nc.dram_tensor("ones_c", [P, 1], BF16, kind="ExternalInput")
    ones1 = nc.dram_tensor("ones1", [1, P], F32R, kind="ExternalInput")
    outT = nc.dram_tensor("outT", [C, T], F32, kind="ExternalOutput")

    scale = 1.0 / np.sqrt(128.0)

    with tile.TileContext(nc) as tc:
        with (
            tc.tile_pool(name="dram", bufs=1, space="DRAM") as dram,
            tc.tile_pool(name="const", bufs=1) as cpool,
        ):
            # per-q-chunk od tiles so phase C's chunk-t read only depends on
            # chunk-t writes; [p, h, t] layout puts the strided cost on the
            # 32 small hidden writes so the C-entry read is contiguous
            od = dram.tile([NQ, P, G, QW], BF16)

            with (
                tc.tile_pool(name="xts", bufs=1) as xts_pool,
                tc.tile_pool(name="kall", bufs=1) as kall_pool,
                tc.tile_pool(name="vall", bufs=1) as va_pool,
            ):
                # resident through phases A and B (all bf16):
                # x^T [ki, ko, t]; roped k [hd, h, t];
                # v_all[ti, to, hh*128+d] = v[to*128+ti, head hh, d]
                # x chunk 0 goes first (the v matmuls need it); consts and
                # the rest alternate between the sync and gpsimd DMA queues
                xtc = [
                    xts_pool.tile([P, KO, QW], BF16, tag=f"xts{c}",
                                  name=f"xts{c}")
                    for c in range(NTC)
                ]
                nc.sync.dma_start(xtc[0][:], xt[0])
                cos_s = cpool.tile([P, T], BF16)
                nc.gpsimd.dma_start(cos_s[:], cosp[:])
                sin_s = cpool.tile([P, T], BF16)
                nc.gpsimd.dma_start(sin_s[:], sinp[:])
                mask_s = cpool.tile([P, P], F32R)
                nc.gpsimd.dma_start(mask_s[:], maskt[:])
                ident_s = cpool.tile([P, P], F32R)
                nc.gpsimd.dma_start(ident_s[:], identt[:])
                one_col = cpool.tile([P, 1], BF16)
                nc.gpsimd.dma_start(one_col[:], ones_c[:])
                one_row = cpool.tile([1, P], F32R)
                nc.gpsimd.dma_start(one_row[:], ones1[:])
                nc.sync.dma_start(xtc[1][:], xt[1])
                nc.gpsimd.dma_start(xtc[2][:], xt[2])
                nc.gpsimd.dma_start(xtc[3][:], xt[3])
                kall = kall_pool.tile([P, G, T], BF16, tag="kall")
                v_all = va_pool.tile([P, NJ, G * P], BF16, tag="vall")

                # --- v (kept in SBUF, natural [t, d] layout) ---
                with (
                    tc.tile_pool(name="wv", bufs=2) as wv_pool,
                    tc.tile_pool(name="psV", bufs=2, space="PSUM") as psV,
                ):
                    for n2 in range(NV):
                        wv_s = wv_pool.tile([P, KO, VN], BF16, tag="wv")
                        nc.scalar.dma_start(wv_s[:], wv[n2])
                        for tb in range(NTB):
                            psv = psV.tile([P, VN], F32, tag="pv")
                            for kc in range(KO):
                                nc.tensor.matmul(
                                    psv[:],
                                    xtc[tb // JPQ][
                                        :, kc,
                                        (tb % JPQ) * P : (tb % JPQ + 1) * P,
                                    ],
                                    wv_s[:, kc, :],
                                    start=(kc == 0),
                                    stop=(kc == KO - 1),
                                )
                            nc.scalar.activation(
                                v_all[:, tb, n2 * VN : (n2 + 1) * VN],
                                psv[:],
                                mybir.ActivationFunctionType.Copy,
                            )

                # --- fused qk projection + RoPE + attention ---
                # per head h: emit qk matmuls+rope for head h, then the
                # attention for head h-1 (1-head lag keeps every PE input
                # ready before the in-order PE queue reaches it).
                with (
                    tc.tile_pool(name="wqk", bufs=2) as wqk_pool,
                    tc.tile_pool(name="qkraw", bufs=1) as qkraw_pool,
                    tc.tile_pool(name="ropf", bufs=2) as ropf_pool,
                    tc.tile_pool(name="sw", bufs=2) as sw_pool,
                    tc.tile_pool(name="ropb", bufs=3) as ropb_pool,
                    tc.tile_pool(name="pt", bufs=5) as pt_pool,
                    tc.tile_pool(name="racc", bufs=2) as racc_pool,
                    tc.tile_pool(name="rsb", bufs=2) as rsb_pool,
                    tc.tile_pool(name="rq", bufs=2) as rq_pool,
                    tc.tile_pool(name="rinv", bufs=2) as rinv_pool,
                    tc.tile_pool(name="oacc", bufs=2) as oacc_pool,
                    tc.tile_pool(name="psSh", bufs=4, space="PSUM") as psSh,
                    tc.tile_pool(name="psO", bufs=3, space="PSUM") as psO,
                    tc.tile_pool(name="psR", bufs=1, space="PSUM") as psR,
                ):
                    qts = {}

                    def emit_qk(m):
                        h, is_k = m // 2, m % 2
                        w_s = wqk_pool.tile([P, KO, P], BF16, tag="wqk")
                        nc.sync.dma_start(w_s[:], wqk[m])
                        raw = qkraw_pool.tile([P, T], BF16, tag="qkraw")
                        ropf = ropf_pool.tile([P, T], BF16, tag="ropf")
                        for c in range(NTC):
                            sl = slice(c * QW, (c + 1) * QW)
                            pss = psSh.tile([P, QW], F32, tag="ps")
                            for kc in range(KO):
                                nc.tensor.matmul(
                                    pss[:],
                                    w_s[:, kc, :],
                                    xtc[c][:, kc, :],
                                    start=(kc == 0),
                                    stop=(kc == KO - 1),
                                    skip_group_check=True,
                                )
                            nc.vector.tensor_copy(raw[:, sl], pss[:])
                            nc.vector.tensor_mul(
                                ropf[:, sl], pss[:], cos_s[:, sl]
                            )
                        # RoPE: rope = raw*[cos;cos] + swap(raw)*[-sin;+sin]
                        sw = sw_pool.tile([P, T], BF16, tag="sw")
                        nc.sync.dma_start(sw[0:64, :], raw[64:128, :])
                        nc.sync.dma_start(sw[64:128, :], raw[0:64, :])
                        nc.vector.tensor_mul(sw[:], sw[:], sin_s[:])
                        if is_k:
                            nc.vector.tensor_add(kall[:, h, :], ropf[:], sw[:])
                        else:
                            rb = ropb_pool.tile([P, T], BF16, tag="ropb")
                            nc.vector.tensor_add(rb[:], ropf[:], sw[:])
                            qts[h] = rb

                    # Normalization tail, three PE-async stages: the rowsum
                    # matmul + DVE reciprocal fire early in the next chunk
                    # (inputs long since ready); the broadcast matmul +
                    # scale + od write fire >=4us of PE work later so the
                    # in-order PE queue never waits on the reciprocal.
                    rs_pend = []
                    norm_pend = []

                    def flush_rs(direct=False):
                        racc_p, pso_p, oT_p, hp, Qp = rs_pend.pop(0)
                        psr = psR.tile([1, QW], F32, tag="psr")
                        nc.tensor.matmul(
                            psr[:], one_col[:], racc_p[:],
                            start=True, stop=True, skip_group_check=True,
                        )
                        rinv1 = rsb_pool.tile([1, QW], F32R, tag="rinv1")
                        if direct:
                            # drain path: shortest latency, DVE cost is moot
                            with nc.allow_low_precision(reason="f32r is 4B"):
                                nc.vector.reciprocal(rinv1[:], psr[:])
                        else:
                            # reciprocal is free-size-bound (~6.5 cyc/elem/
                            # lane): fold [1,512] -> [128,4] via SBUF-SBUF
                            # DMA so it runs on 4 elems/lane (161ns) instead
                            # of 512 (3.3us)
                            rsb = rsb_pool.tile([1, QW], F32, tag="rsb")
                            nc.vector.tensor_copy(rsb[:], psr[:])
                            rq = rq_pool.tile([P, QW // P], F32, tag="rq")
                            nc.sync.dma_start(rq[:], rsb[:])
                            rqr = rq_pool.tile([P, QW // P], F32R, tag="rqr")
                            with nc.allow_low_precision(reason="f32r is 4B"):
                                nc.vector.reciprocal(rqr[:], rq[:])
                            nc.sync.dma_start(rinv1[:], rqr[:])
                        norm_pend.append((rinv1, pso_p, oT_p, hp, Qp))

                    def flush_norm():
                        rinv1, pso_p, oT_p, hp, Qp = norm_pend.pop(0)
                        psrep = psSh.tile([P, QW], F32, tag="ps")
                        nc.tensor.matmul(
                            psrep[:], one_row[:], rinv1[:],
                            start=True, stop=True, skip_group_check=True,
                        )
                        rinv = rinv_pool.tile([P, QW], F32, tag="rinv")
                        nc.scalar.activation(
                            rinv[:], psrep[:],
                            mybir.ActivationFunctionType.Copy,
                        )
                        sl = slice(Qp * QW, (Qp + 1) * QW)
                        nc.vector.tensor_mul(oT_p[:, sl], pso_p[:], rinv[:])
                        nc.scalar.dma_start(od[Qp][:, hp, :], oT_p[:, sl])

                    def emit_attn(h):
                        qT = qts.pop(h)
                        oT = oacc_pool.tile([P, T], BF16, tag="oacc")
                        for Q in range(NQ):
                            jmax = JPQ * (Q + 1) - 1  # inclusive
                            pso = psO.tile([P, QW], F32, tag="pso")
                            racc = racc_pool.tile([P, QW], BF16, tag="racc")
                            # PV(J) consumes exp(S(J)) from ACT; emit it LOOK
                            # S-matmuls later so PE never stalls on ACT.
                            LOOK = 3
                            pend = []
                            for J in range(jmax + 1 + LOOK):
                                if J <= jmax:
                                    k_d = J - JPQ * Q  # diag idx if >= 0
                                    co = max(k_d, 0) * P
                                    pss = psSh.tile([P, QW], F32, tag="ps")
                                    nc.tensor.matmul(
                                        pss[:, co:],
                                        kall[:, h, J * P : (J + 1) * P],
                                        qT[:, Q * QW + co : (Q + 1) * QW],
                                        start=True,
                                        stop=(k_d < 0),
                                        skip_group_check=True,
                                    )
                                    if k_d >= 0:
                                        # causal mask: -1e9 bias, exp -> 0
                                        nc.tensor.matmul(
                                            pss[:, co : co + P],
                                            mask_s[:],
                                            ident_s[:],
                                            start=False,
                                            stop=True,
                                            skip_group_check=True,
                                        )
                                    pT = pt_pool.tile([P, QW], BF16, tag="pt")
                                    nc.scalar.activation(
                                        pT[:, co:], pss[:, co:],
                                        mybir.ActivationFunctionType.Exp,
                                        scale=scale,
                                    )
                                    # rowsum on DVE (bf16): r += exp tile
                                    if J == 0:
                                        nc.vector.tensor_copy(racc[:], pT[:])
                                    else:
                                        nc.vector.tensor_add(
                                            racc[:, co:], racc[:, co:],
                                            pT[:, co:],
                                        )
                                    pend.append((J, co, pT))
                                if J >= LOOK:
                                    Jp, cop, pTp = pend.pop(0)
                                    nc.tensor.matmul(
                                        pso[:, cop:],
                                        v_all[:, Jp, h * P : (h + 1) * P],
                                        pTp[:, cop:],
                                        start=(Jp == 0),
                                        stop=(Jp == jmax),
                                        skip_group_check=True,
                                    )
                                if J == 6 and rs_pend:
                                    flush_rs()
                                if J in (10, 14) and norm_pend:
                                    flush_norm()
                            rs_pend.append((racc, pso, oT, h, Q))

                    for h in range(G):
                        emit_qk(2 * h)
                        # head-boundary rowsum flush lands here: the PE
                        # reaches it a full qk-head after the racc chain of
                        # the previous head finished, so it never stalls
                        if rs_pend:
                            flush_rs()
                        emit_qk(2 * h + 1)
                        if h >= 1:
                            emit_attn(h - 1)
                    emit_attn(G - 1)
                    while rs_pend:
                        flush_rs(direct=True)
                    while norm_pend:
                        flush_norm()

                # ------------- Phase C: output projection -------------
                with (
                    tc.tile_pool(name="wp", bufs=1) as wp_pool,
                    tc.tile_pool(name="otc", bufs=2) as otc_pool,
                    tc.tile_pool(name="csb", bufs=4) as csb_pool,
                    tc.tile_pool(name="psC", bufs=4, space="PSUM") as psC,
                ):
                    wp_s = wp_pool.tile([P, KO, G, P], BF16, tag="wp")
                    for m in range(KO):
                        q = (nc.gpsimd, nc.sync, nc.scalar)[m % 3]
                        q.dma_start(wp_s[:, m], wp[m])
                    for t in range(NQ):
                        oTt = otc_pool.tile([P, G, QW], BF16, tag="otc")
                        nc.sync.dma_start(
                            oTt[:],
                            od[t],
                        )
                        for m in range(KO):
                            psc = psC.tile([P, QW], F32, tag="psc")
                            for h in range(G):
                                nc.tensor.matmul(
                                    psc[:],
                                    wp_s[:, m, h, :],
                                    oTt[:, h, :],
                                    start=(h == 0),
                                    stop=(h == G - 1),
                                )
                            csb = csb_pool.tile([P, QW], F32, tag="csb")
                            nc.any.tensor_copy(csb[:], psc[:])
                            nc.sync.dma_start(
                                outT[
                                    m * P : (m + 1) * P,
                                    t * QW : (t + 1) * QW,
                                ],
                                csb[:],
                            )

    _split_multi_waits(nc)
    return nc


# --------------------------------------------------------------------------
def _prep_core_inputs(xb, w_attn, w_proj, rope_cos, rope_sin, g, G=8):
    """Host-side shard prep for one core: batch slice xb (T, C), group g."""
    from ml_dtypes import bfloat16

    T, C = xb.shape
    KO = C // P
    VN = 512
    NV = (G * P) // VN
    gc = g * G * P  # column offset of this group within one qkv section

    # x^T arranged [chunk, ki, ko, t-in-chunk]: each 512-token chunk is
    # contiguous in DRAM so the chunk DMA moves 16KB lines per partition
    xtT = np.ascontiguousarray(xb.T)  # (C, T)
    xt = np.ascontiguousarray(
        xtT.reshape(KO, P, T // 512, 512).transpose(2, 1, 0, 3)
    )

    # q,k columns for this group, RoPE pair-permuted (2i,2i+1) -> (i,64+i),
    # interleaved m=2h -> q head h, m=2h+1 -> k head h
    perm = np.empty(P, dtype=np.int64)
    perm[:64] = np.arange(0, P, 2)
    perm[64:] = np.arange(1, P, 2)
    wq = w_attn[:, gc : gc + G * P].reshape(C, G, P)[:, :, perm]
    wk = w_attn[:, C + gc : C + gc + G * P].reshape(C, G, P)[:, :, perm]
    wqk_cols = np.stack([wq, wk], axis=2).reshape(C, 2 * G * P)
    wqk = np.ascontiguousarray(
        wqk_cols.reshape(KO, P, 2 * G, P).transpose(2, 1, 0, 3)
    )

    wv_cols = w_attn[:, 2 * C + gc : 2 * C + gc + G * P]  # (C, G*128)
    wv = np.ascontiguousarray(
        wv_cols.reshape(KO, P, NV, VN).transpose(2, 1, 0, 3)
    )

    wp_rows = w_proj[gc : gc + G * P, :]  # (G*128, C)
    wp = np.ascontiguousarray(
        wp_rows.reshape(G, P, KO, P).transpose(2, 1, 0, 3)
    )

    cT = rope_cos[:T].T  # (64, T)
    sT = rope_sin[:T].T
    cospT = np.ascontiguousarray(np.concatenate([cT, cT], axis=0))  # (128, T)
    sinpT = np.ascontiguousarray(np.concatenate([-sT, sT], axis=0))
    # -1e9 bias on the strictly-lower (key > query) triangle of the S^T
    # diagonal block, delivered as lhsT of a bias matmul against identity:
    # pss[jj,qq] += maskb[qq,jj] -> maskb = strict upper triangle.
    maskb = -1e9 * np.triu(np.ones((P, P), dtype=np.float32), k=1)

    return {
        "xt": xt.astype(bfloat16),
        "wqk": wqk.astype(bfloat16),
        "wv": wv.astype(bfloat16),
        "wp": wp.astype(bfloat16),
        "cosp": cospT.astype(bfloat16),
        "sinp": sinpT.astype(bfloat16),
        "maskt": maskb,
        "identt": np.eye(P, dtype=np.float32),
        "ones_c": np.ones((P, 1), dtype=bfloat16),
        "ones1": np.ones((1, P), dtype=np.float32),
    }


_NC_CACHE = {}
TRACE = False
LAST_RESULTS = None


def kernel(x, w_attn, w_proj, rope_cos, rope_sin):
    from concourse.bass_utils import run_bass_kernel_spmd

    x = np.asarray(x, dtype=np.float32)
    w_attn = np.asarray(w_attn, dtype=np.float32)
    w_proj = np.asarray(w_proj, dtype=np.float32)
    rope_cos = np.asarray(rope_cos, dtype=np.float32)
    rope_sin = np.asarray(rope_sin, dtype=np.float32)

    B, T, C = x.shape
    G = 8  # heads per group (16 heads / 2 groups)

    key = (T, C, G)
    if key not in _NC_CACHE:
        _NC_CACHE[key] = build_attention_core(T=T, C=C, G=G)
    nc = _NC_CACHE[key]

    in_maps = []
    for core in range(8):
        b, g = core // 2, core % 2
        in_maps.append(
            _prep_core_inputs(x[b], w_attn, w_proj, rope_cos, rope_sin, g, G=G)
        )

    res = run_bass_kernel_spmd(nc, in_maps, list(range(8)), trace=TRACE)
    global LAST_RESULTS
    LAST_RESULTS = res

    y = np.empty((B, T, C), dtype=np.float32)
    for b in range(B):
        acc = res.results[2 * b]["outT"] + res.results[2 * b + 1]["outT"]
        y[b] = acc.T
    return y
